# revision 11
# baseline (speedup 1.0000x reference)
"""GCN encoder (2-layer GCNConv) on 8 Trainium2 NeuronCores.

Strategy (dest-sharded graph parallel):
  - Destinations sharded by node range across 8 cores (12500 each).
  - Within a core, dests are sorted by in-degree descending. Edges are laid
    out in ELL-prefix "rounds": round t holds the t-th incoming edge of every
    dest whose degree > t. Because dests are degree-sorted, round t touches
    exactly the prefix s[0:n_t] of the accumulator -> the scatter-add becomes
    a contiguous DVE tensor_add; only the gather is data-dependent.
  - Gathers are indirect DMA row-gathers (64B rows) from a replicated
    (AllGather'ed) u-table in DRAM.
  - Both layers aggregate 16-wide features: layer 2 aggregates BEFORE the
    16x64 transform (aggregation commutes with right-multiplication by W2).

out = D^-1/2 (A+I) D^-1/2 relu(D^-1/2 (A+I) D^-1/2 X W1 + b1) W2 + b2
with u = h * dinv:  s[c] = sum_{e: col=c} u[row_e] + u[c];  out_h = s * dinv + b
"""

import math
import sys

import numpy as np

if "/opt/trn_rl_repo" not in sys.path:
    sys.path.insert(0, "/opt/trn_rl_repo")

import concourse.bacc as bacc
import concourse.bass as bass
import concourse.mybir as mybir
import concourse.tile as tile
from concourse import bass_utils
from concourse.masks import make_identity

# ---------------------------------------------------------------- constants
N = 100000
E = 3200000
IN_C, HID, OUT_C = 128, 16, 64
NCORES = 8
SHARD = N // NCORES            # 12500 real dests per core
P = 128
QCH = (SHARD + P - 1) // P     # 98 column-chunks of 128 ranks
SHARD_PAD = QCH * P            # 12544
SFREE = QCH * HID              # 1568 f32 per partition for s
TABLE_ROWS = NCORES * SHARD_PAD  # 100352 rows in the replicated u table
PAD_DEG = 1.0e30               # huge degree for pad ranks -> dinv ~ 1e-15
BCOLS = 256                    # gather-block capacity (columns of 128 rows)

F32 = mybir.dt.float32
I32 = mybir.dt.int32


def _round_profile():
    """Static per-round widths W_t (in 128-rank columns), from the Poisson(32)
    in-degree profile of E uniform edges over N nodes, with an 8-sigma + 64
    margin so any same-distribution input fits. Returns list of W_t."""
    lam = E / N
    # Poisson pmf / sf in float64
    R_MAX = 200
    pmf = np.zeros(R_MAX)
    pmf[0] = math.exp(-lam)
    for k in range(1, R_MAX):
        pmf[k] = pmf[k - 1] * lam / k
    sf = 1.0 - np.cumsum(pmf)  # sf[t] = P(X > t)
    W = []
    for t in range(R_MAX):
        q = max(sf[t], 0.0)
        if N * q < 1e-10 and t > lam:
            break
        nt = SHARD * q
        sig = math.sqrt(max(SHARD * q * (1.0 - q), 0.0))
        w = int(math.ceil((nt + 8.0 * sig + 64.0) / P))
        W.append(max(1, min(QCH, w)))
    W += [1] * 8  # tail insurance for max-degree outliers
    return W


ROUND_W = _round_profile()
NROUNDS = len(ROUND_W)
OFFS_W = sum(ROUND_W)  # total offset columns

# pack whole rounds into gather blocks of <= BCOLS columns
BLOCKS = []  # list of (col_start, ncols, [(round_t, local_col, width)])
_c0, _cur, _curw = 0, [], 0
_col = 0
for _t, _w in enumerate(ROUND_W):
    if _curw + _w > BCOLS and _cur:
        BLOCKS.append((_c0, _curw, _cur))
        _c0, _cur, _curw = _col, [], 0
    _cur.append((_t, _curw, _w))
    _curw += _w
    _col += _w
if _cur:
    BLOCKS.append((_c0, _curw, _cur))


# ---------------------------------------------------------------- device code
def _build_program():
    nc = bacc.Bacc(
        "TRN2",
        target_bir_lowering=False,
        debug=False,
        num_devices=NCORES,
        enable_partition_id=False,
        num_swdge_queues=4,
    )
    xT = nc.dram_tensor("xT", [P, SHARD_PAD], F32, kind="ExternalInput")
    deg_in = nc.dram_tensor("deg", [P, QCH], F32, kind="ExternalInput")
    offs_in = nc.dram_tensor("offs", [P, OFFS_W], I32, kind="ExternalInput")
    w1_in = nc.dram_tensor("W1", [IN_C, HID], F32, kind="ExternalInput")
    w2_in = nc.dram_tensor("W2", [HID, OUT_C], F32, kind="ExternalInput")
    b1_in = nc.dram_tensor("b1", [P, HID], F32, kind="ExternalInput")
    b2_in = nc.dram_tensor("b2", [P, OUT_C], F32, kind="ExternalInput")
    out_d = nc.dram_tensor("out", [SHARD_PAD, OUT_C], F32, kind="ExternalOutput")

    with tile.TileContext(nc) as tc:
        with (
            tc.tile_pool(name="const", bufs=1) as cpool,
            tc.tile_pool(name="work", bufs=1) as wpool,
            tc.tile_pool(name="gath", bufs=2) as gpool,
            tc.tile_pool(name="psum", bufs=3, space="PSUM") as ppool,
            tc.tile_pool(name="psumT", bufs=3, space="PSUM") as ptpool,
            tc.tile_pool(name="dram", bufs=1, space="DRAM") as dpool,
        ):
            # ---- load constants / inputs
            w1_sb = cpool.tile([IN_C, HID], F32, name="w1_sb")
            w2_sb = cpool.tile([HID, OUT_C], F32, name="w2_sb")
            b1_sb = cpool.tile([P, HID], F32, name="b1_sb")
            b2_sb = cpool.tile([P, OUT_C], F32, name="b2_sb")
            ident = cpool.tile([P, P], F32, name="ident")
            deg_sb = cpool.tile([P, QCH], F32, name="deg_sb")
            dinv = cpool.tile([P, QCH], F32, name="dinv")
            offs_sb = cpool.tile([P, OFFS_W], I32, name="offs_sb")
            xT_sb = cpool.tile([P, SHARD_PAD], F32, name="xT_sb")
            u_own = cpool.tile([P, SFREE], F32, name="u_own")
            u2_own = cpool.tile([P, SFREE], F32, name="u2_own")
            s_acc = cpool.tile([P, SFREE], F32, name="s_acc")
            v_sb = cpool.tile([P, SFREE], F32, name="v_sb")
            out_sb = cpool.tile([P, QCH * OUT_C], F32, name="out_sb")

            nc.sync.dma_start(out=w1_sb[:], in_=w1_in[:])
            nc.sync.dma_start(out=w2_sb[:], in_=w2_in[:])
            nc.sync.dma_start(out=b1_sb[:], in_=b1_in[:])
            nc.sync.dma_start(out=b2_sb[:], in_=b2_in[:])
            nc.sync.dma_start(out=deg_sb[:], in_=deg_in[:])
            nc.sync.dma_start(out=offs_sb[:], in_=offs_in[:])
            nc.sync.dma_start(out=xT_sb[:], in_=xT[:])
            make_identity(nc, ident[:])

            # dinv = sqrt(1/deg); pad ranks have deg=1e30 -> dinv ~ 1e-15
            nc.vector.reciprocal(dinv[:], deg_sb[:])
            nc.scalar.activation(dinv[:], dinv[:], mybir.ActivationFunctionType.Sqrt)

            # dinv broadcast over the 16 features of each chunk: [P, QCH, HID]
            def dinv16():
                a = dinv[:]
                return bass.AP(a.tensor, a.offset, [a.ap[0], a.ap[1], [0, HID]])

            dram_u1own = dpool.tile([SHARD_PAD, HID], F32, name="dram_u1own")
            dram_u2own = dpool.tile([SHARD_PAD, HID], F32, name="dram_u2own")
            u1_tab = dpool.tile(
                [TABLE_ROWS, HID], F32, name="u1_tab", addr_space="Shared"
            )
            u2_tab = dpool.tile(
                [TABLE_ROWS, HID], F32, name="u2_tab", addr_space="Shared"
            )

            # ---- layer-1 transform: u1 = (x @ W1) * dinv, chunk by chunk
            for q in range(QCH):
                pt = ppool.tile([P, HID], F32, name="mm1", tag="mm")
                nc.tensor.matmul(
                    out=pt[:],
                    lhsT=xT_sb[:, q * P : (q + 1) * P],
                    rhs=w1_sb[:],
                    start=True,
                    stop=True,
                )
                nc.vector.tensor_scalar(
                    out=u_own[:, q * HID : (q + 1) * HID],
                    in0=pt[:],
                    scalar1=dinv[:, q : q + 1],
                    scalar2=None,
                    op0=mybir.AluOpType.mult,
                )

            # own slice -> DRAM (partition p holds ranks q*128+p => rows p*QCH+q)
            nc.sync.dma_start(
                out=dram_u1own[:].rearrange("(p q) f -> p (q f)", p=P),
                in_=u_own[:],
            )
            nc.gpsimd.collective_compute(
                "AllGather",
                mybir.AluOpType.bypass,
                replica_groups=[list(range(NCORES))],
                ins=[dram_u1own.opt()],
                outs=[u1_tab.opt()],
            )

            # ---- edge aggregation for one layer
            def aggregate(tab, sacc):
                nc.vector.memset(sacc[:], 0.0)
                for c0, ncols, rounds in BLOCKS:
                    g = gpool.tile([P, BCOLS * HID], F32, name="gbuf", tag="gbuf")
                    for lc in range(ncols):
                        nc.gpsimd.indirect_dma_start(
                            out=g[:, lc * HID : (lc + 1) * HID],
                            out_offset=None,
                            in_=tab[:],
                            in_offset=bass.IndirectOffsetOnAxis(
                                ap=offs_sb[:, c0 + lc : c0 + lc + 1], axis=0
                            ),
                        )
                    for _t, lc, w in rounds:
                        nc.vector.tensor_tensor(
                            out=sacc[:, : w * HID],
                            in0=sacc[:, : w * HID],
                            in1=g[:, lc * HID : (lc + w) * HID],
                            op=mybir.AluOpType.add,
                        )

            aggregate(u1_tab, s_acc)

            # self loop + finalize: u2 = relu((s + u1) * dinv + b1) * dinv
            def shaped(t):  # [P, SFREE] -> [P, QCH, HID]
                a = t[:]
                return a.rearrange("p (q f) -> p q f", f=HID)

            def b16(t, f):  # bias tile [P, f] broadcast over chunks
                a = t[:]
                return bass.AP(a.tensor, a.offset, [a.ap[0], [0, QCH], [1, f]])

            nc.vector.tensor_tensor(
                out=s_acc[:], in0=s_acc[:], in1=u_own[:], op=mybir.AluOpType.add
            )
            nc.vector.tensor_tensor(
                out=shaped(s_acc), in0=shaped(s_acc), in1=dinv16(),
                op=mybir.AluOpType.mult,
            )
            nc.vector.tensor_tensor(
                out=shaped(s_acc), in0=shaped(s_acc), in1=b16(b1_sb, HID),
                op=mybir.AluOpType.add,
            )
            nc.scalar.activation(
                s_acc[:], s_acc[:], mybir.ActivationFunctionType.Relu
            )
            nc.vector.tensor_tensor(
                out=shaped(u2_own), in0=shaped(s_acc), in1=dinv16(),
                op=mybir.AluOpType.mult,
            )

            nc.sync.dma_start(
                out=dram_u2own[:].rearrange("(p q) f -> p (q f)", p=P),
                in_=u2_own[:],
            )
            nc.gpsimd.collective_compute(
                "AllGather",
                mybir.AluOpType.bypass,
                replica_groups=[list(range(NCORES))],
                ins=[dram_u2own.opt()],
                outs=[u2_tab.opt()],
            )

            # ---- layer-2 aggregation into v, then out = (v*dinv) @ W2 + b2
            aggregate(u2_tab, v_sb)
            nc.vector.tensor_tensor(
                out=v_sb[:], in0=v_sb[:], in1=u2_own[:], op=mybir.AluOpType.add
            )
            nc.vector.tensor_tensor(
                out=shaped(v_sb), in0=shaped(v_sb), in1=dinv16(),
                op=mybir.AluOpType.mult,
            )

            for q in range(QCH):
                ptt = ptpool.tile([HID, P], F32, name="vT_ps", tag="vT_ps")
                nc.tensor.transpose(
                    out=ptt[:],
                    in_=v_sb[:, q * HID : (q + 1) * HID],
                    identity=ident[:],
                )
                vT = gpool.tile([HID, P], F32, name="vT_sb", tag="vT_sb")
                nc.vector.tensor_copy(out=vT[:], in_=ptt[:])
                po = ppool.tile([P, OUT_C], F32, name="mm2", tag="mm")
                nc.tensor.matmul(
                    out=po[:], lhsT=vT[:], rhs=w2_sb[:], start=True, stop=True
                )
                nc.vector.tensor_tensor(
                    out=out_sb[:, q * OUT_C : (q + 1) * OUT_C],
                    in0=po[:],
                    in1=b2_sb[:],
                    op=mybir.AluOpType.add,
                )

            nc.sync.dma_start(
                out=out_d[:].rearrange("(p q) f -> p (q f)", p=P),
                in_=out_sb[:],
            )

    nc.compile()
    return nc


_NC_CACHE = None


def _get_program():
    global _NC_CACHE
    if _NC_CACHE is None:
        _NC_CACHE = _build_program()
    return _NC_CACHE


# ---------------------------------------------------------------- host prep
def _prep_inputs(x, edge_index, W1, b1, W2, b2):
    """Pure index preprocessing + layout (sharding). Returns in_maps and the
    inverse row permutation for unsharding."""
    x = np.asarray(x, dtype=np.float32)
    row = np.asarray(edge_index[0], dtype=np.int64)
    col = np.asarray(edge_index[1], dtype=np.int64)
    W1 = np.asarray(W1, dtype=np.float32)
    W2 = np.asarray(W2, dtype=np.float32)
    b1 = np.asarray(b1, dtype=np.float32).reshape(-1)
    b2 = np.asarray(b2, dtype=np.float32).reshape(-1)

    indeg = np.bincount(col, minlength=N).astype(np.int64)  # excl self loop
    deg = (indeg + 1).astype(np.float32)

    # per-core rank of each node: sort own range by in-degree descending
    rank = np.empty(N, dtype=np.int64)
    node_of_rank = np.empty((NCORES, SHARD_PAD), dtype=np.int64)
    for c in range(NCORES):
        nodes = np.arange(c * SHARD, (c + 1) * SHARD)
        order = np.argsort(-indeg[nodes], kind="stable")
        rank[nodes[order]] = np.arange(SHARD)
        node_of_rank[c, :SHARD] = nodes[order]
        node_of_rank[c, SHARD:] = -1

    # global u-table row of a node: core*SHARD_PAD + (rank%128)*QCH + rank//128
    core_of = np.arange(N) // SHARD
    table_row = core_of * SHARD_PAD + (rank % P) * QCH + rank // P
    zrow = 0 * SHARD_PAD + (  # core-0 pad rank SHARD_PAD-1 -> guaranteed-zero u
        (SHARD_PAD - 1) % P
    ) * QCH + (SHARD_PAD - 1) // P

    # ELL-prefix round assignment (vectorized): order edges by (core, dest
    # rank, t) where t = within-dest counter
    dcore = col // SHARD
    drank = rank[col]
    ekey = dcore * SHARD_PAD + drank
    eorder = np.argsort(ekey, kind="stable")
    ekey_s = ekey[eorder]
    row_s = row[eorder]
    # within-dest counter t
    starts = np.searchsorted(ekey_s, np.arange(NCORES * SHARD_PAD))
    t_of = np.arange(E) - starts[ekey_s]

    cum_w = np.cumsum([0] + ROUND_W)
    offs_all = np.full((NCORES, P, OFFS_W), zrow, dtype=np.int32)
    dr = ekey_s % SHARD_PAD
    dc = ekey_s // SHARD_PAD
    qq, pp = dr // P, dr % P
    wt = np.asarray(ROUND_W + [0], dtype=np.int64)
    tcl = np.minimum(t_of, len(ROUND_W) - 1)
    ok = (t_of < len(ROUND_W)) & (qq < wt[tcl])
    if not np.all(ok):
        raise RuntimeError(
            "static round profile exceeded: max in-degree or slot overflow"
        )
    offs_all[dc, pp, cum_w[t_of] + qq] = table_row[row_s].astype(np.int32)

    # per-core tensors
    in_maps = []
    b1b = np.broadcast_to(b1, (P, HID)).astype(np.float32).copy()
    b2b = np.broadcast_to(b2, (P, OUT_C)).astype(np.float32).copy()
    for c in range(NCORES):
        nor = node_of_rank[c]
        deg_pi = np.full(SHARD_PAD, PAD_DEG, dtype=np.float32)
        deg_pi[:SHARD] = deg[nor[:SHARD]]
        # [P, QCH] with (p, q) = rank q*128+p
        deg_sb = deg_pi.reshape(QCH, P).T.copy()
        # xT column r = x row of rank r; device chunk q reads cols [q*128, (q+1)*128)
        xT = np.zeros((P, SHARD_PAD), dtype=np.float32)
        xT[:, :SHARD] = x[nor[:SHARD]].T
        in_maps.append(
            {
                "xT": np.ascontiguousarray(xT),
                "deg": np.ascontiguousarray(deg_sb),
                "offs": np.ascontiguousarray(offs_all[c]),
                "W1": W1,
                "W2": W2,
                "b1": b1b,
                "b2": b2b,
            }
        )

    # unshard permutation: out row of node n = core*SHARD_PAD + (r%P)*QCH + r//P
    inv_rows = table_row  # same layout as the concatenated outputs
    return in_maps, inv_rows


def _build_floor_probe():
    """Minimal 8-core program for measuring the PJRT dispatch floor."""
    nc = bacc.Bacc("TRN2", target_bir_lowering=False, debug=False,
                   num_devices=NCORES, enable_partition_id=False)
    a = nc.dram_tensor("a", [P, 16], F32, kind="ExternalInput")
    b = nc.dram_tensor("b", [P, 16], F32, kind="ExternalOutput")
    with tile.TileContext(nc) as tc:
        with tc.tile_pool(name="sb", bufs=1) as sb:
            t = sb.tile([P, 16], F32, name="t")
            nc.sync.dma_start(out=t[:], in_=a[:])
            nc.sync.dma_start(out=b[:], in_=t[:])
    nc.compile()
    return nc


def timed_run(in_maps, reps=5, nc=None):
    """Time device execution of the compiled program (PJRT path, inputs
    pre-staged on device). Returns best wall-ns per execution."""
    import time

    import jax
    from jax.sharding import Mesh, PartitionSpec
    from jax.experimental.shard_map import shard_map as _shard_map

    if nc is None:
        nc = _get_program()
    import concourse.mybir as _mb
    from concourse.bass2jax import _bass_exec_p, install_neuronx_cc_hook

    install_neuronx_cc_hook()
    in_names, out_names, out_avals, zero_outs = [], [], [], []
    for alloc in nc.m.functions[0].allocations:
        if not isinstance(alloc, _mb.MemoryLocationSet):
            continue
        name = alloc.memorylocations[0].name
        if alloc.kind == "ExternalInput":
            in_names.append(name)
        elif alloc.kind == "ExternalOutput":
            out_names.append(name)
            shape = tuple(alloc.tensor_shape)
            dtype = _mb.dt.np(alloc.dtype)
            out_avals.append(jax.core.ShapedArray(shape, dtype))
            zero_outs.append(np.zeros(shape, dtype))
    n_params = len(in_names)
    all_in_names = in_names + out_names

    def _body(*args):
        return tuple(
            _bass_exec_p.bind(
                *args,
                out_avals=tuple(out_avals),
                in_names=tuple(all_in_names),
                out_names=tuple(out_names),
                lowering_input_output_aliases=(),
                sim_require_finite=True,
                sim_require_nnan=True,
                nc=nc,
            )
        )

    devices = jax.devices()[:NCORES]
    mesh = Mesh(np.asarray(devices), ("core",))
    nio = n_params + len(out_names)
    fn = jax.jit(
        _shard_map(
            _body,
            mesh=mesh,
            in_specs=(PartitionSpec("core"),) * nio,
            out_specs=(PartitionSpec("core"),) * len(out_names),
            check_rep=False,
        )
    )
    concat_in = [
        np.concatenate([np.asarray(in_maps[c][nm]) for c in range(NCORES)], axis=0)
        for nm in in_names
    ] + [np.concatenate([z] * NCORES, axis=0) for z in zero_outs]
    sharding = jax.sharding.NamedSharding(mesh, PartitionSpec("core"))
    handles = [jax.device_put(a, sharding) for a in concat_in]
    best = None
    for _ in range(reps):
        t0 = time.perf_counter()
        outs = fn(*handles)
        jax.block_until_ready(outs)
        dt = time.perf_counter() - t0
        if best is None or dt < best:
            best = dt
    return best * 1e9


def kernel(x, edge_index, W1, b1, W2, b2):
    in_maps, inv_rows = _prep_inputs(x, edge_index, W1, b1, W2, b2)
    nc = _get_program()
    res = bass_utils.run_bass_kernel_spmd(
        nc, in_maps, core_ids=list(range(NCORES))
    )
    outs = np.concatenate(
        [res.results[c]["out"] for c in range(NCORES)], axis=0
    )  # [NCORES*SHARD_PAD, OUT_C]
    return np.ascontiguousarray(outs[inv_rows]).astype(np.float32)
